# revision 1
# baseline (speedup 1.0000x reference)
"""Causal self-attention (B=4, T=2048, C=1024, H=16) on 8 trn2 NeuronCores.

Sharding: core = (batch b, head-group g), b in 0..3, g in 0..1. Each core does
8 heads of one batch element (Megatron column split of w_attn, row split of
w_proj); host sums the two partial projection outputs per batch element.

Per-core kernel, v2 (software-pipelined, PE kept dense):
 - All DRAM inputs bf16 (host casts); S-matmul operands stored float32r.
 - Q^T,K^T computed transposed (lhsT=W-block, rhs=x^T-block) so attention
   needs no transposes; V natural with a ones column per head so the
   attention AV matmul accumulates the softmax denominator l for free.
 - Attention per head-pair: S^T for both heads row-tiled into one
   [128,1024] PSUM tile per k-block; one exp (scale=1/8 folded in, no
   max-subtraction -- scores are N(0,1)); causal mask only on diagonal
   blocks via one doubled-mask bf16 multiply; AV deferred one k-block so
   exp latency hides; QK projection matmuls of the NEXT pair interleaved
   as PE filler inside the ACT-bound attention loop.
 - Normalization deferred: l rows gathered, one reciprocal_approx_fast per
   qc batch, rank-1 broadcast matmul + in-place multiply on Y^T; for the
   last pair these (plus the output projection) are the interleaved filler.
"""

import sys

if "/opt/trn_rl_repo" not in sys.path:
    sys.path.insert(0, "/opt/trn_rl_repo")

import numpy as np

T = 2048
C = 1024
G = 512          # per-core head-group width (8 heads x 64)
D = 64           # head dim
NH = 8           # heads per core
E = 65           # augmented head width (64 + ones column)
QCH = 512        # query chunk
KBLK = 128       # key block


def _build_nc():
    from collections import deque
    from contextlib import ExitStack

    import concourse.bass as bass
    import concourse.mybir as mybir
    import concourse.tile as tile
    from concourse import bacc

    F32 = mybir.dt.float32
    F32R = mybir.dt.float32r
    BF16 = mybir.dt.bfloat16
    EXP = mybir.ActivationFunctionType.Exp

    nc = bacc.Bacc("TRN2", target_bir_lowering=False)

    xT = nc.dram_tensor("xT", [C, T], BF16, kind="ExternalInput")
    wq = nc.dram_tensor("wq", [C, G], BF16, kind="ExternalInput")
    wk = nc.dram_tensor("wk", [C, G], BF16, kind="ExternalInput")
    wv = nc.dram_tensor("wv", [C, G], BF16, kind="ExternalInput")
    wp = nc.dram_tensor("wp", [G, C], BF16, kind="ExternalInput")
    mask = nc.dram_tensor("mask", [128, 256], BF16, kind="ExternalInput")
    out = nc.dram_tensor("out", [T, C], F32, kind="ExternalOutput")

    with tile.TileContext(nc) as tc, ExitStack() as ctx:
        persist = ctx.enter_context(tc.tile_pool(name="persist", bufs=1))
        xw = ctx.enter_context(tc.tile_pool(name="xw", bufs=1))
        wsl = ctx.enter_context(tc.tile_pool(name="wsl", bufs=2))
        qtkt = ctx.enter_context(tc.tile_pool(name="qtkt", bufs=2))
        ptp = ctx.enter_context(tc.tile_pool(name="ptp", bufs=4))
        nrm = ctx.enter_context(tc.tile_pool(name="nrm", bufs=2))
        osb = ctx.enter_context(tc.tile_pool(name="osb", bufs=2))
        wpp = ctx.enter_context(tc.tile_pool(name="wpp", bufs=1))
        pss = ctx.enter_context(tc.tile_pool(name="pss", bufs=2, space="PSUM"))
        psy = ctx.enter_context(tc.tile_pool(name="psy", bufs=1, space="PSUM"))
        pfl = ctx.enter_context(tc.tile_pool(name="pfl", bufs=2, space="PSUM"))

        VA = [persist.tile([128, NH * 128], BF16, name=f"va{i}", tag=f"va{i}")
              for i in range(16)]
        YT = [persist.tile([128, T], BF16, name=f"yt{i}", tag=f"yt{i}")
              for i in range(4)]
        MSK = persist.tile([128, 256], BF16, name="msk", tag="msk")
        ones_f32 = persist.tile([128, 64], F32, name="ones_f32", tag="ones_f32")
        ones64 = persist.tile([1, 64], F32R, name="ones64", tag="ones64")
        nc.vector.memset(ones_f32, 1.0)
        nc.vector.tensor_copy(ones64, ones_f32[0:1, :])

        # V weights + first half of xT first: compute can start earliest
        WV = []
        for c in range(8):
            w = wsl.tile([128, G], BF16, name=f"w{c}", tag=f"w{c}")
            nc.sync.dma_start(out=w, in_=wv[c * 128 : (c + 1) * 128, :])
            WV.append(w)
        XT = []
        for c in range(8):
            t = xw.tile([128, T], BF16, name=f"x{c}", tag=f"x{c}")
            nc.sync.dma_start(
                out=t[:, 0:128], in_=xT[c * 128 : (c + 1) * 128, 0:128]
            )
            XT.append(t)
        for c in range(8):
            nc.sync.dma_start(
                out=XT[c][:, 128 : T // 2],
                in_=xT[c * 128 : (c + 1) * 128, 128 : T // 2],
            )
        for c in range(8):
            nc.sync.dma_start(
                out=XT[c][:, T // 2 : T],
                in_=xT[c * 128 : (c + 1) * 128, T // 2 : T],
            )

        # V-augmentation ones columns
        ones_col = ones_f32[:, 0:8].rearrange("p (h o) -> p h o", o=1)
        for tb in range(16):
            vdst = VA[tb].rearrange("p (h e) -> p h e", e=128)[:, :, 64:65]
            nc.vector.tensor_copy(vdst, ones_col)

        # ---------------- phase 0: V ----------------
        for tb in range(16):
            ps = pfl.tile([128, 512], F32, name="fill", tag="fill")
            for c in range(8):
                nc.tensor.matmul(
                    ps,
                    XT[c][:, tb * 128 : (tb + 1) * 128],
                    WV[c],
                    start=(c == 0),
                    stop=(c == 7),
                )
            vdst = VA[tb].rearrange("p (h e) -> p h e", e=128)[:, :, 0:64]
            nc.vector.tensor_copy(vdst, ps.rearrange("p (h d) -> p h d", d=64))

        nc.sync.dma_start(out=MSK, in_=mask[:, :])
        WP = []
        for cb in range(4):
            w = wpp.tile([128, C], BF16, name=f"wpj{cb}", tag=f"wpj{cb}")
            nc.sync.dma_start(out=w, in_=wp[cb * 128 : (cb + 1) * 128, :])
            WP.append(w)

        # ---------------- QK machinery ----------------
        def emit_w_slices(hp):
            tiles = {}
            for mat, dram in (("q", wq), ("k", wk)):
                lst = []
                for c in range(8):
                    w = wsl.tile([128, 128], BF16, name=f"w{c}", tag=f"w{c}")
                    nc.sync.dma_start(
                        out=w,
                        in_=dram[
                            c * 128 : (c + 1) * 128,
                            hp * 128 : (hp + 1) * 128,
                        ],
                    )
                    lst.append(w)
                tiles[mat] = lst
            return tiles

        def make_qk_units(hp):
            wtiles = emit_w_slices(hp)
            qt = qtkt.tile([128, T], BF16, name="qtP", tag="qtP")
            kt = qtkt.tile([128, T], BF16, name="ktP", tag="ktP")
            units = []
            for mat, dst in (("q", qt), ("k", kt)):
                for t4 in range(4):
                    def unit(mat=mat, dst=dst, t4=t4):
                        ps = pfl.tile([128, 512], F32, name="fill", tag="fill")
                        for c in range(8):
                            nc.tensor.matmul(
                                ps,
                                wtiles[mat][c],
                                XT[c][:, t4 * 512 : (t4 + 1) * 512],
                                start=(c == 0),
                                stop=(c == 7),
                            )
                        nc.vector.tensor_copy(
                            dst[:, t4 * 512 : (t4 + 1) * 512], ps
                        )
                    units.append(unit)
            return qt, kt, units

        # ---------- proj units (tail / fillers for pair 3) ----------
        def proj_units(tb):
            ot = {}
            def unit_ch(ch):
                def unit():
                    if ch == 0:
                        ot["t"] = osb.tile([128, C], F32, name="ot", tag="ot")
                    ps = pfl.tile([128, 512], F32, name="fill", tag="fill")
                    for cb in range(4):
                        nc.tensor.matmul(
                            ps,
                            YT[cb][:, tb * 128 : (tb + 1) * 128],
                            WP[cb][:, ch * 512 : (ch + 1) * 512],
                            start=(cb == 0),
                            stop=(cb == 3),
                        )
                    nc.vector.tensor_copy(
                        ot["t"][:, ch * 512 : (ch + 1) * 512], ps
                    )
                    if ch == 1:
                        nc.sync.dma_start(
                            out=out[tb * 128 : (tb + 1) * 128, :], in_=ot["t"]
                        )
                return unit
            return [unit_ch(0), unit_ch(1)]

        def tail_units(qc):
            units = []
            for tb in range(qc * 4, qc * 4 + 4):
                units.extend(proj_units(tb))
            return units

        # ---------------- attention ----------------
        fill_q = deque()

        def pump(n):
            for _ in range(min(n, len(fill_q))):
                fill_q.popleft()()

        def attention(hp, qt, kt, qc):
            q0 = qc * QCH
            nkb = (qc + 1) * 4
            hA, hB = 2 * hp, 2 * hp + 1
            ytA = psy.tile([128, QCH], F32, name="ytA", tag="ytA")
            ytB = psy.tile([128, QCH], F32, name="ytB", tag="ytB")

            def emit_av(kb, pt, off, w):
                nc.tensor.matmul(
                    ytA[:, off : off + w],
                    VA[kb][:, hA * 128 : hA * 128 + 128],
                    pt[:, off : off + w],
                    start=(kb == 0),
                    stop=(kb == nkb - 1),
                )
                nc.tensor.matmul(
                    ytB[:, off : off + w],
                    VA[kb][:, hB * 128 : hB * 128 + 128],
                    pt[:, 512 + off : 512 + off + w],
                    start=(kb == 0),
                    stop=(kb == nkb - 1),
                )

            pend = deque()
            for kb in range(nkb):
                j = kb - qc * 4
                off = j * 128 if j >= 1 else 0
                w = 512 - off
                ksl = slice(kb * KBLK, (kb + 1) * KBLK)
                sAB = pss.tile([128, 1024], F32, name="sAB", tag="sAB")
                nc.tensor.matmul(
                    sAB[:, off : 512],
                    kt[0:64, ksl],
                    qt[0:64, q0 + off : q0 + QCH],
                    start=True,
                    stop=True,
                    tile_position=(0, 0),
                )
                nc.tensor.matmul(
                    sAB[:, 512 + off : 1024],
                    kt[64:128, ksl],
                    qt[64:128, q0 + off : q0 + QCH],
                    start=True,
                    stop=True,
                    tile_position=(64, 0),
                )
                pt = ptp.tile([128, 1024], BF16, name="pt", tag="pt")
                if j >= 1:
                    nc.scalar.activation(
                        pt[:, off:512], sAB[:, off:512], EXP, scale=0.125
                    )
                    nc.scalar.activation(
                        pt[:, 512 + off : 1024],
                        sAB[:, 512 + off : 1024],
                        EXP,
                        scale=0.125,
                    )
                else:
                    nc.scalar.activation(pt, sAB, EXP, scale=0.125)
                if j >= 0:
                    pv = pt.rearrange("p (s q) -> p s q", s=2)[
                        :, :, off : off + 128
                    ]
                    nc.vector.tensor_mul(
                        pv, pv, MSK.rearrange("p (s q) -> p s q", s=2)
                    )
                if kb % 2 == 1 or j >= 0:
                    pump(1)
                if len(pend) == 2:
                    emit_av(*pend.popleft())
                pend.append((kb, pt, off, w))
            while pend:
                emit_av(*pend.popleft())
            for sub, yt in ((0, ytA), (1, ytB)):
                ysl = YT[hp][sub * 64 : (sub + 1) * 64, q0 : q0 + QCH]
                nc.vector.tensor_copy(ysl, yt[0:64, :])
                lf = nrm.tile([1, 512], F32, name="lf", tag="lf")
                nc.vector.tensor_copy(lf, yt[64:65, :])
                lf2 = nrm.tile([1, 512], F32, name="lf2", tag="lf2")
                nc.vector.reciprocal_approx_fast(lf2, lf)
                lr = nrm.tile([1, 512], F32R, name="lr", tag="lr")
                nc.vector.tensor_copy(lr, lf2)

                def norm_fin(ysl=ysl, lr=lr):
                    rb = pfl.tile([64, 512], F32, name="fill", tag="fill")
                    nc.tensor.matmul(rb, ones64, lr, start=True, stop=True)
                    nc.vector.tensor_mul(ysl, ysl, rb)
                fill_q.append(norm_fin)

        # ---------------- main schedule ----------------
        qt, kt, units = make_qk_units(0)
        for u in units:
            u()
        for hp in range(4):
            nqt = nkt = None
            if hp < 3:
                nqt, nkt, nunits = make_qk_units(hp + 1)
                fill_q.extend(nunits)
            for qc in range(4):
                if hp == 3 and qc >= 1:
                    fill_q.extend(tail_units(qc - 1))
                attention(hp, qt, kt, qc)
                pump(2)
            pump(len(fill_q))
            if hp < 3:
                qt, kt = nqt, nkt
        for u in tail_units(3):
            u()

    nc.compile()
    return nc


_NC_CACHE = None


def kernel(x0, w_attn, w_proj, _trace=False, _tmpdir=None):
    global _NC_CACHE
    import ml_dtypes

    from concourse.bass_utils import run_bass_kernel_spmd

    BF = ml_dtypes.bfloat16
    x0 = np.asarray(x0, dtype=np.float32)
    w_attn = np.asarray(w_attn, dtype=np.float32)
    w_proj = np.asarray(w_proj, dtype=np.float32)
    B = x0.shape[0]

    if _NC_CACHE is None:
        _NC_CACHE = _build_nc()
    nc = _NC_CACHE

    tri = np.triu(np.ones((128, 128), dtype=np.float32))
    msk = np.concatenate([tri, tri], axis=1).astype(BF)
    in_maps = []
    for core in range(8):
        b, g = divmod(core, 2)
        in_maps.append(
            {
                "xT": np.ascontiguousarray(x0[b].T).astype(BF),
                "wq": np.ascontiguousarray(
                    w_attn[:, g * G : (g + 1) * G]
                ).astype(BF),
                "wk": np.ascontiguousarray(
                    w_attn[:, C + g * G : C + (g + 1) * G]
                ).astype(BF),
                "wv": np.ascontiguousarray(
                    w_attn[:, 2 * C + g * G : 2 * C + (g + 1) * G]
                ).astype(BF),
                "wp": np.ascontiguousarray(
                    w_proj[g * G : (g + 1) * G, :]
                ).astype(BF),
                "mask": msk,
            }
        )

    res = run_bass_kernel_spmd(
        nc, in_maps, list(range(8)), trace=_trace, tmpdir=_tmpdir
    )
    outp = np.empty((B, T, C), dtype=np.float32)
    for b in range(B):
        outp[b] = res.results[2 * b]["out"] + res.results[2 * b + 1]["out"]
    if _trace:
        kernel.last_exec_time_ns = res.exec_time_ns
    return outp



# revision 5
# speedup vs baseline: 1.0962x; 1.0962x over previous
"""Causal self-attention (B=4, T=2048, C=1024, H=16) on 8 trn2 NeuronCores.

Sharding: core = (batch b, head-group g), b in 0..3, g in 0..1. Each core does
8 heads of one batch element (Megatron column split of w_attn, row split of
w_proj); host sums the two partial projection outputs per batch element.

Per-core kernel, v2 (software-pipelined, PE kept dense):
 - All DRAM inputs bf16 (host casts); S-matmul operands stored float32r.
 - Q^T,K^T computed transposed (lhsT=W-block, rhs=x^T-block) so attention
   needs no transposes; V natural with a ones column per head so the
   attention AV matmul accumulates the softmax denominator l for free.
 - Attention per head-pair: S^T for both heads row-tiled into one
   [128,1024] PSUM tile per k-block; one exp (scale=1/8 folded in, no
   max-subtraction -- scores are N(0,1)); causal mask only on diagonal
   blocks via one doubled-mask bf16 multiply; AV deferred one k-block so
   exp latency hides; QK projection matmuls of the NEXT pair interleaved
   as PE filler inside the ACT-bound attention loop.
 - Normalization deferred: l rows gathered, one reciprocal_approx_fast per
   qc batch, rank-1 broadcast matmul + in-place multiply on Y^T; for the
   last pair these (plus the output projection) are the interleaved filler.
"""

import sys

if "/opt/trn_rl_repo" not in sys.path:
    sys.path.insert(0, "/opt/trn_rl_repo")

import numpy as np

T = 2048
C = 1024
G = 512          # per-core head-group width (8 heads x 64)
D = 64           # head dim
NH = 8           # heads per core
E = 65           # augmented head width (64 + ones column)
QCH = 512        # query chunk
KBLK = 128       # key block


def _build_nc():
    from collections import deque
    from contextlib import ExitStack

    import concourse.bass as bass
    import concourse.mybir as mybir
    import concourse.tile as tile
    from concourse import bacc

    F32 = mybir.dt.float32
    F32R = mybir.dt.float32r
    BF16 = mybir.dt.bfloat16
    EXP = mybir.ActivationFunctionType.Exp

    nc = bacc.Bacc("TRN2", target_bir_lowering=False)

    xT = nc.dram_tensor("xT", [C, T], BF16, kind="ExternalInput")
    wq = nc.dram_tensor("wq", [C, G], BF16, kind="ExternalInput")
    wk = nc.dram_tensor("wk", [C, G], BF16, kind="ExternalInput")
    wv = nc.dram_tensor("wv", [C, G], BF16, kind="ExternalInput")
    wp = nc.dram_tensor("wp", [G, C], BF16, kind="ExternalInput")
    mask = nc.dram_tensor("mask", [128, 256], BF16, kind="ExternalInput")
    out = nc.dram_tensor("out", [T, C], F32, kind="ExternalOutput")

    with tile.TileContext(nc) as tc, ExitStack() as ctx:
        persist = ctx.enter_context(tc.tile_pool(name="persist", bufs=1))
        xw = ctx.enter_context(tc.tile_pool(name="xw", bufs=1))
        wsl = ctx.enter_context(tc.tile_pool(name="wsl", bufs=2))
        qtkt = ctx.enter_context(tc.tile_pool(name="qtkt", bufs=2))
        ptp = ctx.enter_context(tc.tile_pool(name="ptp", bufs=4))
        nrm = ctx.enter_context(tc.tile_pool(name="nrm", bufs=2))
        osb = ctx.enter_context(tc.tile_pool(name="osb", bufs=2))
        wpp = ctx.enter_context(tc.tile_pool(name="wpp", bufs=1))
        pss = ctx.enter_context(tc.tile_pool(name="pss", bufs=2, space="PSUM"))
        psy = ctx.enter_context(tc.tile_pool(name="psy", bufs=1, space="PSUM"))
        pfl = ctx.enter_context(tc.tile_pool(name="pfl", bufs=2, space="PSUM"))

        VA = [persist.tile([128, NH * 128], BF16, name=f"va{i}", tag=f"va{i}")
              for i in range(16)]
        YT = [persist.tile([128, T], BF16, name=f"yt{i}", tag=f"yt{i}")
              for i in range(4)]
        MSK = persist.tile([128, 256], BF16, name="msk", tag="msk")

        # V weights + first half of xT first: compute can start earliest
        WV = []
        for c in range(8):
            w = wsl.tile([128, G], BF16, name=f"w{c}", tag=f"w{c}")
            nc.sync.dma_start(out=w, in_=wv[c * 128 : (c + 1) * 128, :])
            WV.append(w)
        XT = []
        for c in range(8):
            t = xw.tile([128, T], BF16, name=f"x{c}", tag=f"x{c}")
            nc.sync.dma_start(
                out=t[:, 0:128], in_=xT[c * 128 : (c + 1) * 128, 0:128]
            )
            XT.append(t)
        for c in range(8):
            nc.sync.dma_start(
                out=XT[c][:, 128 : T // 2],
                in_=xT[c * 128 : (c + 1) * 128, 128 : T // 2],
            )
        for c in range(8):
            nc.sync.dma_start(
                out=XT[c][:, T // 2 : T],
                in_=xT[c * 128 : (c + 1) * 128, T // 2 : T],
            )

        # V-augmentation: 64 ones columns per head -> AV matmul emits the
        # softmax denominator replicated over psum rows 64:128 for free.
        for tb in range(16):
            vdst = VA[tb].rearrange("p (h e) -> p h e", e=128)[:, :, 64:128]
            nc.vector.memset(vdst, 1.0)

        # ---------------- phase 0: V ----------------
        for tb in range(16):
            ps = pfl.tile([128, 512], F32, name="fill", tag="fill")
            for c in range(8):
                nc.tensor.matmul(
                    ps,
                    XT[c][:, tb * 128 : (tb + 1) * 128],
                    WV[c],
                    start=(c == 0),
                    stop=(c == 7),
                )
            vdst = VA[tb].rearrange("p (h e) -> p h e", e=128)[:, :, 0:64]
            nc.vector.tensor_copy(vdst, ps.rearrange("p (h d) -> p h d", d=64))

        nc.sync.dma_start(out=MSK, in_=mask[:, :])
        WP = []
        for cb in range(4):
            w = wpp.tile([128, C], BF16, name=f"wpj{cb}", tag=f"wpj{cb}")
            nc.sync.dma_start(out=w, in_=wp[cb * 128 : (cb + 1) * 128, :])
            WP.append(w)

        # ---------------- QK machinery ----------------
        def emit_w_slices(hp):
            tiles = {}
            for mat, dram in (("q", wq), ("k", wk)):
                lst = []
                for c in range(8):
                    w = wsl.tile([128, 128], BF16, name=f"w{c}", tag=f"w{c}")
                    nc.sync.dma_start(
                        out=w,
                        in_=dram[
                            c * 128 : (c + 1) * 128,
                            hp * 128 : (hp + 1) * 128,
                        ],
                    )
                    lst.append(w)
                tiles[mat] = lst
            return tiles

        def make_qk_units(hp):
            wtiles = emit_w_slices(hp)
            qt = qtkt.tile([128, T], BF16, name="qtP", tag="qtP")
            kt = qtkt.tile([128, T], BF16, name="ktP", tag="ktP")
            units = []
            for mat, dst in (("q", qt), ("k", kt)):
                for t4 in range(4):
                    def unit(mat=mat, dst=dst, t4=t4):
                        ps = pfl.tile([128, 512], F32, name="fill", tag="fill")
                        for c in range(8):
                            nc.tensor.matmul(
                                ps,
                                wtiles[mat][c],
                                XT[c][:, t4 * 512 : (t4 + 1) * 512],
                                start=(c == 0),
                                stop=(c == 7),
                            )
                        nc.vector.tensor_copy(
                            dst[:, t4 * 512 : (t4 + 1) * 512], ps
                        )
                    units.append(unit)
            return qt, kt, units

        # ---------- proj units (tail / fillers for pair 3) ----------
        def proj_units(tb):
            ot = {}
            def unit_ch(ch):
                def unit():
                    if ch == 0:
                        ot["t"] = osb.tile([128, C], F32, name="ot", tag="ot")
                    ps = pfl.tile([128, 512], F32, name="fill", tag="fill")
                    for cb in range(4):
                        nc.tensor.matmul(
                            ps,
                            YT[cb][:, tb * 128 : (tb + 1) * 128],
                            WP[cb][:, ch * 512 : (ch + 1) * 512],
                            start=(cb == 0),
                            stop=(cb == 3),
                        )
                    nc.vector.tensor_copy(
                        ot["t"][:, ch * 512 : (ch + 1) * 512], ps
                    )
                    if ch == 1:
                        nc.sync.dma_start(
                            out=out[tb * 128 : (tb + 1) * 128, :], in_=ot["t"]
                        )
                return unit
            return [unit_ch(0), unit_ch(1)]

        def tail_units(qc):
            units = []
            for tb in range(qc * 4, qc * 4 + 4):
                units.extend(proj_units(tb))
            return units

        # ---------------- attention ----------------
        fill_q = deque()

        def pump(n):
            for _ in range(min(n, len(fill_q))):
                fill_q.popleft()()

        def attention(hp, qt, kt, qc):
            q0 = qc * QCH
            nkb = (qc + 1) * 4
            hA, hB = 2 * hp, 2 * hp + 1
            ytA = psy.tile([128, QCH], F32, name="ytA", tag="ytA")
            ytB = psy.tile([128, QCH], F32, name="ytB", tag="ytB")

            def emit_av(kb, pt, off, w):
                nc.tensor.matmul(
                    ytA[:, off : off + w],
                    VA[kb][:, hA * 128 : hA * 128 + 128],
                    pt[:, off : off + w],
                    start=(kb == 0),
                    stop=(kb == nkb - 1),
                )
                nc.tensor.matmul(
                    ytB[:, off : off + w],
                    VA[kb][:, hB * 128 : hB * 128 + 128],
                    pt[:, 512 + off : 512 + off + w],
                    start=(kb == 0),
                    stop=(kb == nkb - 1),
                )

            pend = deque()
            for kb in range(nkb):
                j = kb - qc * 4
                off = j * 128 if j >= 1 else 0
                w = 512 - off
                ksl = slice(kb * KBLK, (kb + 1) * KBLK)
                sAB = pss.tile([128, 1024], F32, name="sAB", tag="sAB")
                nc.tensor.matmul(
                    sAB[:, off : 512],
                    kt[0:64, ksl],
                    qt[0:64, q0 + off : q0 + QCH],
                    start=True,
                    stop=True,
                    tile_position=(0, 0),
                )
                nc.tensor.matmul(
                    sAB[:, 512 + off : 1024],
                    kt[64:128, ksl],
                    qt[64:128, q0 + off : q0 + QCH],
                    start=True,
                    stop=True,
                    tile_position=(64, 0),
                )
                pt = ptp.tile([128, 1024], BF16, name="pt", tag="pt")
                if j >= 1:
                    nc.scalar.activation(
                        pt[:, off:512], sAB[:, off:512], EXP, scale=0.125
                    )
                    nc.scalar.activation(
                        pt[:, 512 + off : 1024],
                        sAB[:, 512 + off : 1024],
                        EXP,
                        scale=0.125,
                    )
                else:
                    nc.scalar.activation(pt, sAB, EXP, scale=0.125)
                if j >= 0:
                    pv = pt.rearrange("p (s q) -> p s q", s=2)[
                        :, :, off : off + 128
                    ]
                    nc.vector.tensor_mul(
                        pv, pv, MSK.rearrange("p (s q) -> p s q", s=2)
                    )
                if kb % 2 == 1 or j >= 0:
                    pump(1)
                if len(pend) == 2:
                    emit_av(*pend.popleft())
                pend.append((kb, pt, off, w))
            while pend:
                emit_av(*pend.popleft())
            for sub, yt in ((0, ytA), (1, ytB)):
                ysl = YT[hp][sub * 64 : (sub + 1) * 64, q0 : q0 + QCH]
                lsb = nrm.tile([64, 512], F32, name="lsb", tag="lsb")
                nc.vector.tensor_copy(lsb, yt[64:128, :])
                lrec = nrm.tile([64, 512], F32, name="lrec", tag="lrec")
                nc.vector.reciprocal_approx_fast(lrec, lsb)
                nc.vector.tensor_mul(ysl, yt[0:64, :], lrec)

        # ---------------- main schedule ----------------
        qt, kt, units = make_qk_units(0)
        for u in units:
            u()
        for hp in range(4):
            nqt = nkt = None
            if hp < 3:
                nqt, nkt, nunits = make_qk_units(hp + 1)
                fill_q.extend(nunits)
            for qc in range(4):
                if hp == 3 and qc >= 1:
                    fill_q.extend(tail_units(qc - 1))
                attention(hp, qt, kt, qc)
                pump(2)
            pump(len(fill_q))
            if hp < 3:
                qt, kt = nqt, nkt
        for u in tail_units(3):
            u()

    nc.compile()
    return nc


_NC_CACHE = None


def kernel(x0, w_attn, w_proj, _trace=False, _tmpdir=None):
    global _NC_CACHE
    import ml_dtypes

    from concourse.bass_utils import run_bass_kernel_spmd

    BF = ml_dtypes.bfloat16
    x0 = np.asarray(x0, dtype=np.float32)
    w_attn = np.asarray(w_attn, dtype=np.float32)
    w_proj = np.asarray(w_proj, dtype=np.float32)
    B = x0.shape[0]

    if _NC_CACHE is None:
        _NC_CACHE = _build_nc()
    nc = _NC_CACHE

    tri = np.triu(np.ones((128, 128), dtype=np.float32))
    msk = np.concatenate([tri, tri], axis=1).astype(BF)
    in_maps = []
    for core in range(8):
        b, g = divmod(core, 2)
        in_maps.append(
            {
                "xT": np.ascontiguousarray(x0[b].T).astype(BF),
                "wq": np.ascontiguousarray(
                    w_attn[:, g * G : (g + 1) * G]
                ).astype(BF),
                "wk": np.ascontiguousarray(
                    w_attn[:, C + g * G : C + (g + 1) * G]
                ).astype(BF),
                "wv": np.ascontiguousarray(
                    w_attn[:, 2 * C + g * G : 2 * C + (g + 1) * G]
                ).astype(BF),
                "wp": np.ascontiguousarray(
                    w_proj[g * G : (g + 1) * G, :]
                ).astype(BF),
                "mask": msk,
            }
        )

    res = run_bass_kernel_spmd(
        nc, in_maps, list(range(8)), trace=_trace, tmpdir=_tmpdir
    )
    outp = np.empty((B, T, C), dtype=np.float32)
    for b in range(B):
        outp[b] = res.results[2 * b]["out"] + res.results[2 * b + 1]["out"]
    if _trace:
        kernel.last_exec_time_ns = res.exec_time_ns
    return outp



# revision 8
# speedup vs baseline: 1.1111x; 1.0136x over previous
"""Causal self-attention (B=4, T=2048, C=1024, H=16) on 8 trn2 NeuronCores.

Sharding: core = (batch b, head-group g), b in 0..3, g in 0..1. Each core does
8 heads of one batch element (Megatron column split of w_attn, row split of
w_proj); host sums the two partial projection outputs per batch element.

Per-core kernel, v2 (software-pipelined, PE kept dense):
 - All DRAM inputs bf16 (host casts); S-matmul operands stored float32r.
 - Q^T,K^T computed transposed (lhsT=W-block, rhs=x^T-block) so attention
   needs no transposes; V natural with a ones column per head so the
   attention AV matmul accumulates the softmax denominator l for free.
 - Attention per head-pair: S^T for both heads row-tiled into one
   [128,1024] PSUM tile per k-block; one exp (scale=1/8 folded in, no
   max-subtraction -- scores are N(0,1)); causal mask only on diagonal
   blocks via one doubled-mask bf16 multiply; AV deferred one k-block so
   exp latency hides; QK projection matmuls of the NEXT pair interleaved
   as PE filler inside the ACT-bound attention loop.
 - Normalization deferred: l rows gathered, one reciprocal_approx_fast per
   qc batch, rank-1 broadcast matmul + in-place multiply on Y^T; for the
   last pair these (plus the output projection) are the interleaved filler.
"""

import sys

if "/opt/trn_rl_repo" not in sys.path:
    sys.path.insert(0, "/opt/trn_rl_repo")

import numpy as np

T = 2048
C = 1024
G = 512          # per-core head-group width (8 heads x 64)
D = 64           # head dim
NH = 8           # heads per core
E = 65           # augmented head width (64 + ones column)
QCH = 512        # query chunk
KBLK = 128       # key block


def _build_nc():
    from collections import deque
    from contextlib import ExitStack

    import concourse.bass as bass
    import concourse.mybir as mybir
    import concourse.tile as tile
    from concourse import bacc

    F32 = mybir.dt.float32
    F32R = mybir.dt.float32r
    BF16 = mybir.dt.bfloat16
    EXP = mybir.ActivationFunctionType.Exp

    nc = bacc.Bacc("TRN2", target_bir_lowering=False)

    xT = nc.dram_tensor("xT", [C, T], BF16, kind="ExternalInput")
    wq = nc.dram_tensor("wq", [C, G], BF16, kind="ExternalInput")
    wk = nc.dram_tensor("wk", [C, G], BF16, kind="ExternalInput")
    wv = nc.dram_tensor("wv", [C, G], BF16, kind="ExternalInput")
    wp = nc.dram_tensor("wp", [G, C], BF16, kind="ExternalInput")
    mask = nc.dram_tensor("mask", [128, 256], BF16, kind="ExternalInput")
    out = nc.dram_tensor("out", [T, C], F32, kind="ExternalOutput")

    with tile.TileContext(nc) as tc, ExitStack() as ctx:
        persist = ctx.enter_context(tc.tile_pool(name="persist", bufs=1))
        xw = ctx.enter_context(tc.tile_pool(name="xw", bufs=1))
        wsl = ctx.enter_context(tc.tile_pool(name="wsl", bufs=2))
        qtkt = ctx.enter_context(tc.tile_pool(name="qtkt", bufs=2))
        ptp = ctx.enter_context(tc.tile_pool(name="ptp", bufs=6))
        nrm = ctx.enter_context(tc.tile_pool(name="nrm", bufs=2))
        osb = ctx.enter_context(tc.tile_pool(name="osb", bufs=2))
        wpp = ctx.enter_context(tc.tile_pool(name="wpp", bufs=1))
        pss = ctx.enter_context(tc.tile_pool(name="pss", bufs=2, space="PSUM"))
        psy = ctx.enter_context(tc.tile_pool(name="psy", bufs=1, space="PSUM"))
        pfl = ctx.enter_context(tc.tile_pool(name="pfl", bufs=2, space="PSUM"))

        VA = [persist.tile([128, NH * 128], BF16, name=f"va{i}", tag=f"va{i}")
              for i in range(16)]
        YT = [persist.tile([128, T], BF16, name=f"yt{i}", tag=f"yt{i}")
              for i in range(4)]
        MSK = persist.tile([128, 256], BF16, name="msk", tag="msk")

        # V weights + first half of xT first: compute can start earliest
        WV = []
        for c in range(8):
            w = wsl.tile([128, G], BF16, name=f"w{c}", tag=f"w{c}")
            nc.sync.dma_start(out=w, in_=wv[c * 128 : (c + 1) * 128, :])
            WV.append(w)
        XT = []
        for c in range(8):
            t = xw.tile([128, T], BF16, name=f"x{c}", tag=f"x{c}")
            nc.sync.dma_start(
                out=t[:, 0:128], in_=xT[c * 128 : (c + 1) * 128, 0:128]
            )
            XT.append(t)
        for c in range(8):
            nc.sync.dma_start(
                out=XT[c][:, 128 : T // 2],
                in_=xT[c * 128 : (c + 1) * 128, 128 : T // 2],
            )
        for c in range(8):
            nc.sync.dma_start(
                out=XT[c][:, T // 2 : T],
                in_=xT[c * 128 : (c + 1) * 128, T // 2 : T],
            )

        # V-augmentation: 64 ones columns per head -> AV matmul emits the
        # softmax denominator replicated over psum rows 64:128 for free.
        for tb in range(16):
            vdst = VA[tb].rearrange("p (h e) -> p h e", e=128)[:, :, 64:128]
            nc.vector.memset(vdst, 1.0)

        # ---------------- phase 0: V ----------------
        for tb in range(16):
            ps = pfl.tile([128, 512], F32, name="fill", tag="fill")
            for c in range(8):
                nc.tensor.matmul(
                    ps,
                    XT[c][:, tb * 128 : (tb + 1) * 128],
                    WV[c],
                    start=(c == 0),
                    stop=(c == 7),
                )
            vdst = VA[tb].rearrange("p (h e) -> p h e", e=128)[:, :, 0:64]
            nc.vector.tensor_copy(vdst, ps.rearrange("p (h d) -> p h d", d=64))

        nc.sync.dma_start(out=MSK, in_=mask[:, :])
        WP = []
        for cb in range(4):
            w = wpp.tile([128, C], BF16, name=f"wpj{cb}", tag=f"wpj{cb}")
            nc.sync.dma_start(out=w, in_=wp[cb * 128 : (cb + 1) * 128, :])
            WP.append(w)

        # ---------------- QK machinery ----------------
        def emit_w_slices(hp):
            tiles = {}
            for mat, dram in (("q", wq), ("k", wk)):
                lst = []
                for c in range(8):
                    w = wsl.tile([128, 128], BF16, name=f"w{c}", tag=f"w{c}")
                    nc.sync.dma_start(
                        out=w,
                        in_=dram[
                            c * 128 : (c + 1) * 128,
                            hp * 128 : (hp + 1) * 128,
                        ],
                    )
                    lst.append(w)
                tiles[mat] = lst
            return tiles

        def make_qk_units(hp):
            wtiles = emit_w_slices(hp)
            qt = qtkt.tile([128, T], BF16, name="qtP", tag="qtP")
            kt = qtkt.tile([128, T], BF16, name="ktP", tag="ktP")
            units = []
            for t4 in range(4):
                for mat, dst in (("q", qt), ("k", kt)):
                    st = {}
                    def unit_a(mat=mat, t4=t4, st=st):
                        st["ps"] = pfl.tile(
                            [128, 512], F32, name="fill", tag="fill"
                        )
                        for c in range(4):
                            nc.tensor.matmul(
                                st["ps"],
                                wtiles[mat][c],
                                XT[c][:, t4 * 512 : (t4 + 1) * 512],
                                start=(c == 0),
                                stop=False,
                            )
                    def unit_b(mat=mat, dst=dst, t4=t4, st=st):
                        for c in range(4, 8):
                            nc.tensor.matmul(
                                st["ps"],
                                wtiles[mat][c],
                                XT[c][:, t4 * 512 : (t4 + 1) * 512],
                                start=False,
                                stop=(c == 7),
                            )
                        nc.vector.tensor_copy(
                            dst[:, t4 * 512 : (t4 + 1) * 512], st["ps"]
                        )
                    units.append(unit_a)
                    units.append(unit_b)
            return qt, kt, units

        # ---------- proj units (tail / fillers for pair 3) ----------
        def proj_units(tb):
            ot = {}
            def unit_ch(ch):
                def unit():
                    if ch == 0:
                        ot["t"] = osb.tile([128, C], F32, name="ot", tag="ot")
                    ps = pfl.tile([128, 512], F32, name="fill", tag="fill")
                    for cb in range(4):
                        nc.tensor.matmul(
                            ps,
                            YT[cb][:, tb * 128 : (tb + 1) * 128],
                            WP[cb][:, ch * 512 : (ch + 1) * 512],
                            start=(cb == 0),
                            stop=(cb == 3),
                        )
                    nc.vector.tensor_copy(
                        ot["t"][:, ch * 512 : (ch + 1) * 512], ps
                    )
                    if ch == 1:
                        nc.sync.dma_start(
                            out=out[tb * 128 : (tb + 1) * 128, :], in_=ot["t"]
                        )
                return unit
            return [unit_ch(0), unit_ch(1)]

        def tail_units(qc):
            units = []
            for tb in range(qc * 4, qc * 4 + 4):
                units.extend(proj_units(tb))
            return units

        # ---------------- attention ----------------
        fill_q = deque()

        def pump(n):
            for _ in range(min(n, len(fill_q))):
                fill_q.popleft()()

        def attention(hp, qt, kt, qc):
            q0 = qc * QCH
            nkb = (qc + 1) * 4
            hA, hB = 2 * hp, 2 * hp + 1
            ytA = psy.tile([128, QCH], F32, name="ytA", tag="ytA")
            ytB = psy.tile([128, QCH], F32, name="ytB", tag="ytB")

            def emit_av(kb, pt, off, w):
                nc.tensor.matmul(
                    ytA[:, off : off + w],
                    VA[kb][:, hA * 128 : hA * 128 + 128],
                    pt[:, off : off + w],
                    start=(kb == 0),
                    stop=(kb == nkb - 1),
                )
                nc.tensor.matmul(
                    ytB[:, off : off + w],
                    VA[kb][:, hB * 128 : hB * 128 + 128],
                    pt[:, 512 + off : 512 + off + w],
                    start=(kb == 0),
                    stop=(kb == nkb - 1),
                )

            def emit_s(kb):
                j = kb - qc * 4
                off = j * 128 if j >= 1 else 0
                ksl = slice(kb * KBLK, (kb + 1) * KBLK)
                sAB = pss.tile([128, 1024], F32, name="sAB", tag="sAB")
                nc.tensor.matmul(
                    sAB[:, off : 512],
                    kt[0:64, ksl],
                    qt[0:64, q0 + off : q0 + QCH],
                    start=True,
                    stop=True,
                    tile_position=(0, 0),
                )
                nc.tensor.matmul(
                    sAB[:, 512 + off : 1024],
                    kt[64:128, ksl],
                    qt[64:128, q0 + off : q0 + QCH],
                    start=True,
                    stop=True,
                    tile_position=(64, 0),
                )
                return kb, sAB, off

            def emit_exp(kb, sAB, off):
                j = kb - qc * 4
                pt = ptp.tile([128, 1024], BF16, name="pt", tag="pt")
                if j >= 1:
                    nc.scalar.activation(
                        pt[:, off:512], sAB[:, off:512], EXP, scale=0.125
                    )
                    nc.scalar.activation(
                        pt[:, 512 + off : 1024],
                        sAB[:, 512 + off : 1024],
                        EXP,
                        scale=0.125,
                    )
                else:
                    nc.scalar.activation(pt, sAB, EXP, scale=0.125)
                if j >= 0:
                    pv = pt.rearrange("p (s q) -> p s q", s=2)[
                        :, :, off : off + 128
                    ]
                    nc.vector.tensor_mul(
                        pv, pv, MSK.rearrange("p (s q) -> p s q", s=2)
                    )
                return (kb, pt, off, 512 - off)

            pend = deque()
            for kbp in range(0, nkb, 2):
                s0 = emit_s(kbp)
                s1 = emit_s(kbp + 1)
                e0 = emit_exp(*s0)
                e1 = emit_exp(*s1)
                pump(2)
                if len(pend) == 4:
                    emit_av(*pend.popleft())
                    emit_av(*pend.popleft())
                pend.append(e0)
                pend.append(e1)
            while pend:
                emit_av(*pend.popleft())
            for sub, yt in ((0, ytA), (1, ytB)):
                ysl = YT[hp][sub * 64 : (sub + 1) * 64, q0 : q0 + QCH]
                lsb = nrm.tile([64, 512], F32, name="lsb", tag="lsb")
                nc.vector.tensor_copy(lsb, yt[64:128, :])
                lrec = nrm.tile([64, 512], F32, name="lrec", tag="lrec")
                nc.vector.reciprocal_approx_fast(lrec, lsb)
                nc.vector.tensor_mul(ysl, yt[0:64, :], lrec)

        # ---------------- main schedule ----------------
        qt, kt, units = make_qk_units(0)
        for u in units:
            u()
        for hp in range(4):
            nqt = nkt = None
            if hp < 3:
                nqt, nkt, nunits = make_qk_units(hp + 1)
                fill_q.extend(nunits)
            for qc in range(4):
                if hp == 3 and qc >= 1:
                    fill_q.extend(tail_units(qc - 1))
                attention(hp, qt, kt, qc)
                pump(2)
            pump(len(fill_q))
            if hp < 3:
                qt, kt = nqt, nkt
        for u in tail_units(3):
            u()

    nc.compile()
    return nc


_NC_CACHE = None


def kernel(x0, w_attn, w_proj, _trace=False, _tmpdir=None):
    global _NC_CACHE
    import ml_dtypes

    from concourse.bass_utils import run_bass_kernel_spmd

    BF = ml_dtypes.bfloat16
    x0 = np.asarray(x0, dtype=np.float32)
    w_attn = np.asarray(w_attn, dtype=np.float32)
    w_proj = np.asarray(w_proj, dtype=np.float32)
    B = x0.shape[0]

    if _NC_CACHE is None:
        _NC_CACHE = _build_nc()
    nc = _NC_CACHE

    tri = np.triu(np.ones((128, 128), dtype=np.float32))
    msk = np.concatenate([tri, tri], axis=1).astype(BF)
    in_maps = []
    for core in range(8):
        b, g = divmod(core, 2)
        in_maps.append(
            {
                "xT": np.ascontiguousarray(x0[b].T).astype(BF),
                "wq": np.ascontiguousarray(
                    w_attn[:, g * G : (g + 1) * G]
                ).astype(BF),
                "wk": np.ascontiguousarray(
                    w_attn[:, C + g * G : C + (g + 1) * G]
                ).astype(BF),
                "wv": np.ascontiguousarray(
                    w_attn[:, 2 * C + g * G : 2 * C + (g + 1) * G]
                ).astype(BF),
                "wp": np.ascontiguousarray(
                    w_proj[g * G : (g + 1) * G, :]
                ).astype(BF),
                "mask": msk,
            }
        )

    res = run_bass_kernel_spmd(
        nc, in_maps, list(range(8)), trace=_trace, tmpdir=_tmpdir
    )
    outp = np.empty((B, T, C), dtype=np.float32)
    for b in range(B):
        outp[b] = res.results[2 * b]["out"] + res.results[2 * b + 1]["out"]
    if _trace:
        kernel.last_exec_time_ns = res.exec_time_ns
    return outp



# revision 17
# speedup vs baseline: 1.1620x; 1.0458x over previous
"""Causal self-attention (B=4, T=2048, C=1024, H=16) on 8 trn2 NeuronCores.

Sharding: core = (batch b, head-group g), b in 0..3, g in 0..1. Each core does
8 heads of one batch element (Megatron column split of w_attn, row split of
w_proj); host sums the two partial projection outputs per batch element.

Per-core kernel, v2 (software-pipelined, PE kept dense):
 - All DRAM inputs bf16 (host casts); S-matmul operands stored float32r.
 - Q^T,K^T computed transposed (lhsT=W-block, rhs=x^T-block) so attention
   needs no transposes; V natural with a ones column per head so the
   attention AV matmul accumulates the softmax denominator l for free.
 - Attention per head-pair: S^T for both heads row-tiled into one
   [128,1024] PSUM tile per k-block; one exp (scale=1/8 folded in, no
   max-subtraction -- scores are N(0,1)); causal mask only on diagonal
   blocks via one doubled-mask bf16 multiply; AV deferred one k-block so
   exp latency hides; QK projection matmuls of the NEXT pair interleaved
   as PE filler inside the ACT-bound attention loop.
 - Normalization deferred: l rows gathered, one reciprocal_approx_fast per
   qc batch, rank-1 broadcast matmul + in-place multiply on Y^T; for the
   last pair these (plus the output projection) are the interleaved filler.
"""

import sys

if "/opt/trn_rl_repo" not in sys.path:
    sys.path.insert(0, "/opt/trn_rl_repo")

import numpy as np

T = 2048
C = 1024
G = 512          # per-core head-group width (8 heads x 64)
D = 64           # head dim
NH = 8           # heads per core
E = 65           # augmented head width (64 + ones column)
QCH = 512        # query chunk
KBLK = 128       # key block


def _build_nc():
    from collections import deque
    from contextlib import ExitStack

    import concourse.bass as bass
    import concourse.mybir as mybir
    import concourse.tile as tile
    from concourse import bacc

    F32 = mybir.dt.float32
    F32R = mybir.dt.float32r
    BF16 = mybir.dt.bfloat16
    EXP = mybir.ActivationFunctionType.Exp

    nc = bacc.Bacc("TRN2", target_bir_lowering=False)

    # All inputs host-packed so every DMA is one instruction with large
    # per-partition-contiguous runs (cheap descriptor generation).
    # xT: 4 column-waves, each [128p, 8c, cols] flattened per partition.
    xT = nc.dram_tensor("xT", [128, 8 * T], BF16, kind="ExternalInput")
    wq = nc.dram_tensor("wq", [4, 128, 8 * 128], BF16, kind="ExternalInput")
    wk = nc.dram_tensor("wk", [4, 128, 8 * 128], BF16, kind="ExternalInput")
    wv = nc.dram_tensor("wv", [128, 8 * G], BF16, kind="ExternalInput")
    wp = nc.dram_tensor("wp", [128, 4 * C], BF16, kind="ExternalInput")
    mask = nc.dram_tensor("mask", [128, 256], BF16, kind="ExternalInput")
    out = nc.dram_tensor("out", [T, C], F32, kind="ExternalOutput")

    with tile.TileContext(nc) as tc, ExitStack() as ctx:
        persist = ctx.enter_context(tc.tile_pool(name="persist", bufs=1))
        xw = ctx.enter_context(tc.tile_pool(name="xw", bufs=1))
        wsl = ctx.enter_context(tc.tile_pool(name="wsl", bufs=2))
        qtkt = ctx.enter_context(tc.tile_pool(name="qtkt", bufs=2))
        ptp = ctx.enter_context(tc.tile_pool(name="ptp", bufs=6))
        nrm = ctx.enter_context(tc.tile_pool(name="nrm", bufs=2))
        osb = ctx.enter_context(tc.tile_pool(name="osb", bufs=2))
        wpp = ctx.enter_context(tc.tile_pool(name="wpp", bufs=1))
        pss = ctx.enter_context(tc.tile_pool(name="pss", bufs=2, space="PSUM"))
        psy = ctx.enter_context(tc.tile_pool(name="psy", bufs=1, space="PSUM"))
        pfl = ctx.enter_context(tc.tile_pool(name="pfl", bufs=2, space="PSUM"))

        VA = [persist.tile([128, NH * 128], BF16, name=f"va{i}", tag=f"va{i}")
              for i in range(16)]
        YT = [persist.tile([128, T], BF16, name=f"yt{i}", tag=f"yt{i}")
              for i in range(4)]
        MSK = persist.tile([128, 256], BF16, name="msk", tag="msk")

        # V weights + xT column-waves: compute can start earliest
        XTA = xw.tile([128, 8, T], BF16, name="xta", tag="xta")
        nc.sync.dma_start(out=XTA[:, :, 0:128], in_=xT[:, 0:1024])
        WVA = wsl.tile([128, 8, G], BF16, name="wva", tag="wva")
        nc.sync.dma_start(out=WVA, in_=wv[:, :])
        nc.sync.dma_start(out=XTA[:, :, 128:512], in_=xT[:, 1024:4096])
        nc.sync.dma_start(out=XTA[:, :, 512:1024], in_=xT[:, 4096:8192])
        nc.sync.dma_start(out=XTA[:, :, 1024:2048], in_=xT[:, 8192:16384])

        # V-augmentation: 64 ones columns per head -> AV matmul emits the
        # softmax denominator replicated over psum rows 64:128 for free.
        for tb in range(16):
            vdst = VA[tb].rearrange("p (h e) -> p h e", e=128)[:, :, 64:128]
            nc.vector.memset(vdst, 1.0)

        # ---------------- phase 0: V ----------------
        for tb in range(16):
            ps = pfl.tile([128, 512], F32, name="fill", tag="fill")
            for c in range(8):
                nc.tensor.matmul(
                    ps,
                    XTA[:, c, tb * 128 : (tb + 1) * 128],
                    WVA[:, c, :],
                    start=(c == 0),
                    stop=(c == 7),
                )
            vdst = VA[tb].rearrange("p (h e) -> p h e", e=128)[:, :, 0:64]
            nc.vector.tensor_copy(vdst, ps.rearrange("p (h d) -> p h d", d=64))

        nc.sync.dma_start(out=MSK, in_=mask[:, :])
        WPA = wpp.tile([128, 4, C], BF16, name="wpa", tag="wpa")
        nc.sync.dma_start(out=WPA, in_=wp[:, :])

        # ---------------- QK machinery ----------------
        def emit_w_slices(hp):
            tiles = {}
            for mat, dram in (("q", wq), ("k", wk)):
                w = wsl.tile(
                    [128, 8, 128], BF16, name=f"w{mat}", tag=f"w{mat}"
                )
                nc.sync.dma_start(out=w, in_=dram[hp, :, :])
                tiles[mat] = w
            return tiles

        def make_qk_units(hp):
            wtiles = emit_w_slices(hp)
            qt = qtkt.tile([128, T], BF16, name="qtP", tag="qtP")
            kt = qtkt.tile([128, T], BF16, name="ktP", tag="ktP")
            units = []
            for t4 in range(4):
                for mat, dst in (("q", qt), ("k", kt)):
                    st = {}
                    def unit_a(mat=mat, t4=t4, st=st):
                        st["ps"] = pfl.tile(
                            [128, 512], F32, name="fill", tag="fill"
                        )
                        for c in range(4):
                            nc.tensor.matmul(
                                st["ps"],
                                wtiles[mat][:, c, :],
                                XTA[:, c, t4 * 512 : (t4 + 1) * 512],
                                start=(c == 0),
                                stop=False,
                            )
                    def unit_b(mat=mat, dst=dst, t4=t4, st=st):
                        for c in range(4, 8):
                            nc.tensor.matmul(
                                st["ps"],
                                wtiles[mat][:, c, :],
                                XTA[:, c, t4 * 512 : (t4 + 1) * 512],
                                start=False,
                                stop=(c == 7),
                            )
                        nc.vector.tensor_copy(
                            dst[:, t4 * 512 : (t4 + 1) * 512], st["ps"]
                        )
                    units.append(unit_a)
                    units.append(unit_b)
            return qt, kt, units

        # ---------- proj units (tail / fillers for pair 3) ----------
        def proj_units(tb):
            ot = {}
            def unit_ch(ch):
                def unit():
                    if ch == 0:
                        ot["t"] = osb.tile([128, C], F32, name="ot", tag="ot")
                    ps = pfl.tile([128, 512], F32, name="fill", tag="fill")
                    for cb in range(4):
                        nc.tensor.matmul(
                            ps,
                            YT[cb][:, tb * 128 : (tb + 1) * 128],
                            WPA[:, cb, ch * 512 : (ch + 1) * 512],
                            start=(cb == 0),
                            stop=(cb == 3),
                        )
                    nc.vector.tensor_copy(
                        ot["t"][:, ch * 512 : (ch + 1) * 512], ps
                    )
                    if ch == 1:
                        nc.sync.dma_start(
                            out=out[tb * 128 : (tb + 1) * 128, :], in_=ot["t"]
                        )
                return unit
            return [unit_ch(0), unit_ch(1)]

        def tail_units(qc):
            units = []
            for tb in range(qc * 4, qc * 4 + 4):
                units.extend(proj_units(tb))
            return units

        # ---------------- attention ----------------
        fill_q = deque()

        def pump(n):
            for _ in range(min(n, len(fill_q))):
                fill_q.popleft()()

        def attention(hp, qt, kt, qc):
            q0 = qc * QCH
            nkb = (qc + 1) * 4
            hA, hB = 2 * hp, 2 * hp + 1
            ytA = psy.tile([128, QCH], F32, name="ytA", tag="ytA")
            ytB = psy.tile([128, QCH], F32, name="ytB", tag="ytB")

            def emit_av(kb, pt, off, w):
                nc.tensor.matmul(
                    ytA[:, off : off + w],
                    VA[kb][:, hA * 128 : hA * 128 + 128],
                    pt[:, off : off + w],
                    start=(kb == 0),
                    stop=(kb == nkb - 1),
                )
                nc.tensor.matmul(
                    ytB[:, off : off + w],
                    VA[kb][:, hB * 128 : hB * 128 + 128],
                    pt[:, 512 + off : 512 + off + w],
                    start=(kb == 0),
                    stop=(kb == nkb - 1),
                )

            def emit_s(kb):
                j = kb - qc * 4
                off = j * 128 if j >= 1 else 0
                ksl = slice(kb * KBLK, (kb + 1) * KBLK)
                sAB = pss.tile([128, 1024], F32, name="sAB", tag="sAB")
                nc.tensor.matmul(
                    sAB[:, off : 512],
                    kt[0:64, ksl],
                    qt[0:64, q0 + off : q0 + QCH],
                    start=True,
                    stop=True,
                    tile_position=(0, 0),
                )
                nc.tensor.matmul(
                    sAB[:, 512 + off : 1024],
                    kt[64:128, ksl],
                    qt[64:128, q0 + off : q0 + QCH],
                    start=True,
                    stop=True,
                    tile_position=(64, 0),
                )
                return kb, sAB, off

            def emit_exp(kb, sAB, off):
                j = kb - qc * 4
                pt = ptp.tile([128, 1024], BF16, name="pt", tag="pt")
                if j >= 1:
                    nc.scalar.activation(
                        pt[:, off:512], sAB[:, off:512], EXP, scale=0.125
                    )
                    nc.scalar.activation(
                        pt[:, 512 + off : 1024],
                        sAB[:, 512 + off : 1024],
                        EXP,
                        scale=0.125,
                    )
                else:
                    nc.scalar.activation(pt, sAB, EXP, scale=0.125)
                if j >= 0:
                    pv = pt.rearrange("p (s q) -> p s q", s=2)[
                        :, :, off : off + 128
                    ]
                    nc.vector.tensor_mul(
                        pv, pv, MSK.rearrange("p (s q) -> p s q", s=2)
                    )
                return (kb, pt, off, 512 - off)

            pend = deque()
            for kbp in range(0, nkb, 2):
                s0 = emit_s(kbp)
                s1 = emit_s(kbp + 1)
                e0 = emit_exp(*s0)
                e1 = emit_exp(*s1)
                pump(2)
                if len(pend) == 4:
                    emit_av(*pend.popleft())
                    emit_av(*pend.popleft())
                pend.append(e0)
                pend.append(e1)
            while pend:
                emit_av(*pend.popleft())
            for sub, yt in ((0, ytA), (1, ytB)):
                ysl = YT[hp][sub * 64 : (sub + 1) * 64, q0 : q0 + QCH]
                lsb = nrm.tile([64, 512], F32, name="lsb", tag="lsb")
                nc.vector.tensor_copy(lsb, yt[64:128, :])
                lrec = nrm.tile([64, 512], F32, name="lrec", tag="lrec")
                nc.vector.reciprocal_approx_fast(lrec, lsb)
                nc.vector.tensor_mul(ysl, yt[0:64, :], lrec)

        # ---------------- main schedule ----------------
        qt, kt, units = make_qk_units(0)
        for u in units:
            u()
        for hp in range(4):
            nqt = nkt = None
            if hp < 3:
                nqt, nkt, nunits = make_qk_units(hp + 1)
                fill_q.extend(nunits)
            for qc in range(4):
                if hp == 3 and qc >= 1:
                    fill_q.extend(tail_units(qc - 1))
                attention(hp, qt, kt, qc)
                pump(2)
            pump(len(fill_q))
            if hp < 3:
                qt, kt = nqt, nkt
        for u in tail_units(3):
            u()

    nc.compile()
    return nc


_NC_CACHE = None


def kernel(x0, w_attn, w_proj, _trace=False, _tmpdir=None):
    global _NC_CACHE
    import ml_dtypes

    from concourse.bass_utils import run_bass_kernel_spmd

    BF = ml_dtypes.bfloat16
    x0 = np.asarray(x0, dtype=np.float32)
    w_attn = np.asarray(w_attn, dtype=np.float32)
    w_proj = np.asarray(w_proj, dtype=np.float32)
    B = x0.shape[0]

    if _NC_CACHE is None:
        _NC_CACHE = _build_nc()
    nc = _NC_CACHE

    tri = np.triu(np.ones((128, 128), dtype=np.float32))
    msk = np.concatenate([tri, tri], axis=1).astype(BF)

    def pack_x(xb):
        # [C, T] -> [128p, 8c, T] -> 4 column-waves flattened per partition
        xw = xb.T.reshape(8, 128, T).transpose(1, 0, 2)
        waves = [xw[:, :, 0:128], xw[:, :, 128:512],
                 xw[:, :, 512:1024], xw[:, :, 1024:2048]]
        return np.ascontiguousarray(
            np.concatenate([w.reshape(128, -1) for w in waves], axis=1)
        ).astype(BF)

    def pack_qk(wm):
        # [C, G] -> [4hp, 128p, 8c*128] per-pair contiguous
        return np.ascontiguousarray(
            wm.reshape(8, 128, 4, 128).transpose(2, 1, 0, 3).reshape(
                4, 128, 1024
            )
        ).astype(BF)

    in_maps = []
    for core in range(8):
        b, g = divmod(core, 2)
        wvg = w_attn[:, 2 * C + g * G : 2 * C + (g + 1) * G]
        wpg = w_proj[g * G : (g + 1) * G, :]
        in_maps.append(
            {
                "xT": pack_x(x0[b]),
                "wq": pack_qk(w_attn[:, g * G : (g + 1) * G]),
                "wk": pack_qk(w_attn[:, C + g * G : C + (g + 1) * G]),
                "wv": np.ascontiguousarray(
                    wvg.reshape(8, 128, G).transpose(1, 0, 2).reshape(
                        128, 8 * G
                    )
                ).astype(BF),
                "wp": np.ascontiguousarray(
                    wpg.reshape(4, 128, C).transpose(1, 0, 2).reshape(
                        128, 4 * C
                    )
                ).astype(BF),
                "mask": msk,
            }
        )

    res = run_bass_kernel_spmd(
        nc, in_maps, list(range(8)), trace=_trace, tmpdir=_tmpdir
    )
    outp = np.empty((B, T, C), dtype=np.float32)
    for b in range(B):
        outp[b] = res.results[2 * b]["out"] + res.results[2 * b + 1]["out"]
    if _trace:
        kernel.last_exec_time_ns = res.exec_time_ns
    return outp



# revision 24
# speedup vs baseline: 1.1725x; 1.0090x over previous
"""Causal self-attention (B=4, T=2048, C=1024, H=16) on 8 trn2 NeuronCores.

Sharding: core = (batch b, head-group g), b in 0..3, g in 0..1. Each core does
8 heads of one batch element (Megatron column split of w_attn, row split of
w_proj); host sums the two partial projection outputs per batch element.

Per-core kernel, v2 (software-pipelined, PE kept dense):
 - All DRAM inputs bf16 (host casts); S-matmul operands stored float32r.
 - Q^T,K^T computed transposed (lhsT=W-block, rhs=x^T-block) so attention
   needs no transposes; V natural with a ones column per head so the
   attention AV matmul accumulates the softmax denominator l for free.
 - Attention per head-pair: S^T for both heads row-tiled into one
   [128,1024] PSUM tile per k-block; one exp (scale=1/8 folded in, no
   max-subtraction -- scores are N(0,1)); causal mask only on diagonal
   blocks via one doubled-mask bf16 multiply; AV deferred one k-block so
   exp latency hides; QK projection matmuls of the NEXT pair interleaved
   as PE filler inside the ACT-bound attention loop.
 - Normalization deferred: l rows gathered, one reciprocal_approx_fast per
   qc batch, rank-1 broadcast matmul + in-place multiply on Y^T; for the
   last pair these (plus the output projection) are the interleaved filler.
"""

import sys

if "/opt/trn_rl_repo" not in sys.path:
    sys.path.insert(0, "/opt/trn_rl_repo")

import numpy as np

T = 2048
C = 1024
G = 512          # per-core head-group width (8 heads x 64)
D = 64           # head dim
NH = 8           # heads per core
E = 65           # augmented head width (64 + ones column)
QCH = 512        # query chunk
KBLK = 128       # key block


def _build_nc():
    from collections import deque
    from contextlib import ExitStack

    import concourse.bass as bass
    import concourse.mybir as mybir
    import concourse.tile as tile
    from concourse import bacc

    F32 = mybir.dt.float32
    F32R = mybir.dt.float32r
    BF16 = mybir.dt.bfloat16
    EXP = mybir.ActivationFunctionType.Exp

    nc = bacc.Bacc("TRN2", target_bir_lowering=False)

    # All inputs host-packed so every DMA is one instruction with large
    # per-partition-contiguous runs (big descriptors run at full DMA rate).
    # boot: x columns 0:128 (c-major) ++ all of WV -- the critical startup
    # set in one 10KB/partition-descriptor DMA.
    boot = nc.dram_tensor("boot", [128, 1024 + 8 * G], BF16,
                          kind="ExternalInput")
    # xT: 3 column-waves [0:512],[512:1024],[1024:2048], [128p, 8c, cols]
    # flattened per partition (cols 0:128 duplicated in boot).
    xT = nc.dram_tensor("xT", [128, 8 * T], BF16, kind="ExternalInput")
    wq = nc.dram_tensor("wq", [4, 128, 8 * 128], BF16, kind="ExternalInput")
    wk = nc.dram_tensor("wk", [4, 128, 8 * 128], BF16, kind="ExternalInput")
    wp = nc.dram_tensor("wp", [128, 4 * C], BF16, kind="ExternalInput")
    mask = nc.dram_tensor("mask", [128, 256], BF16, kind="ExternalInput")
    out = nc.dram_tensor("out", [T, C], BF16, kind="ExternalOutput")

    with tile.TileContext(nc) as tc, ExitStack() as ctx:
        persist = ctx.enter_context(tc.tile_pool(name="persist", bufs=1))
        xw = ctx.enter_context(tc.tile_pool(name="xw", bufs=1))
        wsl = ctx.enter_context(tc.tile_pool(name="wsl", bufs=2))
        qtkt = ctx.enter_context(tc.tile_pool(name="qtkt", bufs=2))
        ptp = ctx.enter_context(tc.tile_pool(name="ptp", bufs=6))
        nrm = ctx.enter_context(tc.tile_pool(name="nrm", bufs=2))
        osb = ctx.enter_context(tc.tile_pool(name="osb", bufs=2))
        wpp = ctx.enter_context(tc.tile_pool(name="wpp", bufs=1))
        pss = ctx.enter_context(tc.tile_pool(name="pss", bufs=2, space="PSUM"))
        psy = ctx.enter_context(tc.tile_pool(name="psy", bufs=1, space="PSUM"))
        pfl = ctx.enter_context(tc.tile_pool(name="pfl", bufs=2, space="PSUM"))

        VA = [persist.tile([128, NH * 128], BF16, name=f"va{i}", tag=f"va{i}")
              for i in range(16)]
        YT = [persist.tile([128, T], BF16, name=f"yt{i}", tag=f"yt{i}")
              for i in range(4)]
        MSK = persist.tile([128, 256], BF16, name="msk", tag="msk")

        # Boot DMA (x wave0 + WV fused) first, then xT column-waves.
        BOOT = wsl.tile([128, 1024 + 8 * G], BF16, name="boot", tag="boot")
        nc.sync.dma_start(out=BOOT, in_=boot[:, :])
        WVA = BOOT[:, 1024 : 1024 + 8 * G].rearrange("p (c n) -> p c n", c=8)
        XTA = xw.tile([128, 8, T], BF16, name="xta", tag="xta")
        nc.sync.dma_start(out=XTA[:, :, 0:512], in_=xT[:, 0:4096])
        nc.sync.dma_start(out=XTA[:, :, 512:1024], in_=xT[:, 4096:8192])
        nc.sync.dma_start(out=XTA[:, :, 1024:2048], in_=xT[:, 8192:16384])

        # V-augmentation: 64 ones columns per head -> AV matmul emits the
        # softmax denominator replicated over psum rows 64:128 for free.
        for tb in range(16):
            vdst = VA[tb].rearrange("p (h e) -> p h e", e=128)[:, :, 64:128]
            nc.vector.memset(vdst, 1.0)

        # ---------------- phase 0: V ----------------
        for tb in range(16):
            ps = pfl.tile([128, 512], F32, name="fill", tag="fill")
            for c in range(8):
                lhsT = (
                    BOOT[:, c * 128 : (c + 1) * 128]
                    if tb == 0
                    else XTA[:, c, tb * 128 : (tb + 1) * 128]
                )
                nc.tensor.matmul(
                    ps,
                    lhsT,
                    WVA[:, c, :],
                    start=(c == 0),
                    stop=(c == 7),
                )
            vdst = VA[tb].rearrange("p (h e) -> p h e", e=128)[:, :, 0:64]
            nc.vector.tensor_copy(vdst, ps.rearrange("p (h d) -> p h d", d=64))

        nc.sync.dma_start(out=MSK, in_=mask[:, :])
        WPA = wpp.tile([128, 4, C], BF16, name="wpa", tag="wpa")
        nc.sync.dma_start(out=WPA, in_=wp[:, :])

        # ---------------- QK machinery ----------------
        def emit_w_slices(hp):
            tiles = {}
            for mat, dram in (("q", wq), ("k", wk)):
                w = wsl.tile(
                    [128, 8, 128], BF16, name=f"w{mat}", tag=f"w{mat}"
                )
                nc.sync.dma_start(out=w, in_=dram[hp, :, :])
                tiles[mat] = w
            return tiles

        def make_qk_units(hp):
            wtiles = emit_w_slices(hp)
            qt = qtkt.tile([128, T], BF16, name="qtP", tag="qtP")
            kt = qtkt.tile([128, T], BF16, name="ktP", tag="ktP")
            units = []
            for t4 in range(4):
                for mat, dst in (("q", qt), ("k", kt)):
                    st = {}
                    def unit_a(mat=mat, t4=t4, st=st):
                        st["ps"] = pfl.tile(
                            [128, 512], F32, name="fill", tag="fill"
                        )
                        for c in range(4):
                            nc.tensor.matmul(
                                st["ps"],
                                wtiles[mat][:, c, :],
                                XTA[:, c, t4 * 512 : (t4 + 1) * 512],
                                start=(c == 0),
                                stop=False,
                            )
                    def unit_b(mat=mat, dst=dst, t4=t4, st=st):
                        for c in range(4, 8):
                            nc.tensor.matmul(
                                st["ps"],
                                wtiles[mat][:, c, :],
                                XTA[:, c, t4 * 512 : (t4 + 1) * 512],
                                start=False,
                                stop=(c == 7),
                            )
                        nc.vector.tensor_copy(
                            dst[:, t4 * 512 : (t4 + 1) * 512], st["ps"]
                        )
                    units.append(unit_a)
                    units.append(unit_b)
            return qt, kt, units

        # ---------- proj units (tail / fillers for pair 3) ----------
        def proj_units(tb):
            ot = {}
            def unit_ch(ch):
                def unit():
                    if ch == 0:
                        ot["t"] = osb.tile([128, C], BF16, name="ot", tag="ot")
                    ps = pfl.tile([128, 512], F32, name="fill", tag="fill")
                    for cb in range(4):
                        nc.tensor.matmul(
                            ps,
                            YT[cb][:, tb * 128 : (tb + 1) * 128],
                            WPA[:, cb, ch * 512 : (ch + 1) * 512],
                            start=(cb == 0),
                            stop=(cb == 3),
                        )
                    nc.vector.tensor_copy(
                        ot["t"][:, ch * 512 : (ch + 1) * 512], ps
                    )
                    nc.sync.dma_start(
                        out=out[
                            tb * 128 : (tb + 1) * 128,
                            ch * 512 : (ch + 1) * 512,
                        ],
                        in_=ot["t"][:, ch * 512 : (ch + 1) * 512],
                    )
                return unit
            return [unit_ch(0), unit_ch(1)]

        def tail_units(qc):
            units = []
            for tb in range(qc * 4, qc * 4 + 4):
                units.extend(proj_units(tb))
            return units

        # ---------------- attention ----------------
        fill_q = deque()

        def pump(n):
            for _ in range(min(n, len(fill_q))):
                fill_q.popleft()()

        def attention(hp, qt, kt, qc):
            q0 = qc * QCH
            nkb = (qc + 1) * 4
            hA, hB = 2 * hp, 2 * hp + 1
            ytA = psy.tile([128, QCH], F32, name="ytA", tag="ytA")
            ytB = psy.tile([128, QCH], F32, name="ytB", tag="ytB")

            def emit_av(kb, pt, off, w):
                nc.tensor.matmul(
                    ytA[:, off : off + w],
                    VA[kb][:, hA * 128 : hA * 128 + 128],
                    pt[:, off : off + w],
                    start=(kb == 0),
                    stop=(kb == nkb - 1),
                )
                nc.tensor.matmul(
                    ytB[:, off : off + w],
                    VA[kb][:, hB * 128 : hB * 128 + 128],
                    pt[:, 512 + off : 512 + off + w],
                    start=(kb == 0),
                    stop=(kb == nkb - 1),
                )

            def emit_s(kb):
                j = kb - qc * 4
                off = j * 128 if j >= 1 else 0
                ksl = slice(kb * KBLK, (kb + 1) * KBLK)
                sAB = pss.tile([128, 1024], F32, name="sAB", tag="sAB")
                nc.tensor.matmul(
                    sAB[:, off : 512],
                    kt[0:64, ksl],
                    qt[0:64, q0 + off : q0 + QCH],
                    start=True,
                    stop=True,
                    tile_position=(0, 0),
                )
                nc.tensor.matmul(
                    sAB[:, 512 + off : 1024],
                    kt[64:128, ksl],
                    qt[64:128, q0 + off : q0 + QCH],
                    start=True,
                    stop=True,
                    tile_position=(64, 0),
                )
                return kb, sAB, off

            def emit_exp(kb, sAB, off):
                j = kb - qc * 4
                pt = ptp.tile([128, 1024], BF16, name="pt", tag="pt")
                if j >= 1:
                    nc.scalar.activation(
                        pt[:, off:512], sAB[:, off:512], EXP, scale=0.125
                    )
                    nc.scalar.activation(
                        pt[:, 512 + off : 1024],
                        sAB[:, 512 + off : 1024],
                        EXP,
                        scale=0.125,
                    )
                else:
                    nc.scalar.activation(pt, sAB, EXP, scale=0.125)
                if j >= 0:
                    pv = pt.rearrange("p (s q) -> p s q", s=2)[
                        :, :, off : off + 128
                    ]
                    nc.vector.tensor_mul(
                        pv, pv, MSK.rearrange("p (s q) -> p s q", s=2)
                    )
                return (kb, pt, off, 512 - off)

            pend = deque()
            for kbp in range(0, nkb, 2):
                s0 = emit_s(kbp)
                s1 = emit_s(kbp + 1)
                e0 = emit_exp(*s0)
                e1 = emit_exp(*s1)
                pump(2)
                if len(pend) == 4:
                    emit_av(*pend.popleft())
                    emit_av(*pend.popleft())
                pend.append(e0)
                pend.append(e1)
            while pend:
                emit_av(*pend.popleft())
            for sub, yt in ((0, ytA), (1, ytB)):
                ysl = YT[hp][sub * 64 : (sub + 1) * 64, q0 : q0 + QCH]
                lsb = nrm.tile([64, 512], F32, name="lsb", tag="lsb")
                nc.vector.tensor_copy(lsb, yt[64:128, :])
                lrec = nrm.tile([64, 512], F32, name="lrec", tag="lrec")
                nc.vector.reciprocal_approx_fast(lrec, lsb)
                nc.vector.tensor_mul(ysl, yt[0:64, :], lrec)

        # ---------------- main schedule ----------------
        qt, kt, units = make_qk_units(0)
        for u in units:
            u()
        for hp in range(4):
            nqt = nkt = None
            if hp < 3:
                nqt, nkt, nunits = make_qk_units(hp + 1)
                fill_q.extend(nunits)
            for qc in range(4):
                if hp == 3 and qc >= 1:
                    fill_q.extend(tail_units(qc - 1))
                attention(hp, qt, kt, qc)
                pump(2)
            pump(len(fill_q))
            if hp < 3:
                qt, kt = nqt, nkt
        for u in tail_units(3):
            u()

    nc.compile()
    return nc


_NC_CACHE = None


def kernel(x0, w_attn, w_proj, _trace=False, _tmpdir=None):
    global _NC_CACHE
    import ml_dtypes

    from concourse.bass_utils import run_bass_kernel_spmd

    BF = ml_dtypes.bfloat16
    x0 = np.asarray(x0, dtype=np.float32)
    w_attn = np.asarray(w_attn, dtype=np.float32)
    w_proj = np.asarray(w_proj, dtype=np.float32)
    B = x0.shape[0]

    if _NC_CACHE is None:
        _NC_CACHE = _build_nc()
    nc = _NC_CACHE

    tri = np.triu(np.ones((128, 128), dtype=np.float32))
    msk = np.concatenate([tri, tri], axis=1).astype(BF)

    def pack_x(xb):
        # [C, T] -> [128p, 8c, T] -> 3 column-waves flattened per partition
        xw = xb.T.reshape(8, 128, T).transpose(1, 0, 2)
        waves = [xw[:, :, 0:512], xw[:, :, 512:1024], xw[:, :, 1024:2048]]
        return np.ascontiguousarray(
            np.concatenate([w.reshape(128, -1) for w in waves], axis=1)
        ).astype(BF)

    def pack_boot(xb, wvg):
        # x cols 0:128 (c-major per partition) ++ WV (c-major per partition)
        xw = xb.T.reshape(8, 128, T).transpose(1, 0, 2)[:, :, 0:128]
        wvw = wvg.reshape(8, 128, G).transpose(1, 0, 2)
        return np.ascontiguousarray(
            np.concatenate(
                [xw.reshape(128, -1), wvw.reshape(128, -1)], axis=1
            )
        ).astype(BF)

    def pack_qk(wm):
        # [C, G] -> [4hp, 128p, 8c*128] per-pair contiguous
        return np.ascontiguousarray(
            wm.reshape(8, 128, 4, 128).transpose(2, 1, 0, 3).reshape(
                4, 128, 1024
            )
        ).astype(BF)

    in_maps = []
    for core in range(8):
        b, g = divmod(core, 2)
        wvg = w_attn[:, 2 * C + g * G : 2 * C + (g + 1) * G]
        wpg = w_proj[g * G : (g + 1) * G, :]
        in_maps.append(
            {
                "boot": pack_boot(x0[b], wvg),
                "xT": pack_x(x0[b]),
                "wq": pack_qk(w_attn[:, g * G : (g + 1) * G]),
                "wk": pack_qk(w_attn[:, C + g * G : C + (g + 1) * G]),
                "wp": np.ascontiguousarray(
                    wpg.reshape(4, 128, C).transpose(1, 0, 2).reshape(
                        128, 4 * C
                    )
                ).astype(BF),
                "mask": msk,
            }
        )

    res = run_bass_kernel_spmd(
        nc, in_maps, list(range(8)), trace=_trace, tmpdir=_tmpdir
    )
    outp = np.empty((B, T, C), dtype=np.float32)
    for b in range(B):
        outp[b] = res.results[2 * b]["out"].astype(np.float32) + res.results[
            2 * b + 1
        ]["out"].astype(np.float32)
    if _trace:
        kernel.last_exec_time_ns = res.exec_time_ns
    return outp



# revision 30
# speedup vs baseline: 1.1745x; 1.0018x over previous
"""Causal self-attention (B=4, T=2048, C=1024, H=16) on 8 trn2 NeuronCores.

Sharding: core = (batch b, head-group g), b in 0..3, g in 0..1. Each core does
8 heads of one batch element (Megatron column split of w_attn, row split of
w_proj); host sums the two partial projection outputs per batch element.

Per-core kernel, v2 (software-pipelined, PE kept dense):
 - All DRAM inputs bf16 (host casts); S-matmul operands stored float32r.
 - Q^T,K^T computed transposed (lhsT=W-block, rhs=x^T-block) so attention
   needs no transposes; V natural with a ones column per head so the
   attention AV matmul accumulates the softmax denominator l for free.
 - Attention per head-pair: S^T for both heads row-tiled into one
   [128,1024] PSUM tile per k-block; one exp (scale=1/8 folded in, no
   max-subtraction -- scores are N(0,1)); causal mask only on diagonal
   blocks via one doubled-mask bf16 multiply; AV deferred one k-block so
   exp latency hides; QK projection matmuls of the NEXT pair interleaved
   as PE filler inside the ACT-bound attention loop.
 - Normalization deferred: l rows gathered, one reciprocal_approx_fast per
   qc batch, rank-1 broadcast matmul + in-place multiply on Y^T; for the
   last pair these (plus the output projection) are the interleaved filler.
"""

import sys

if "/opt/trn_rl_repo" not in sys.path:
    sys.path.insert(0, "/opt/trn_rl_repo")

import numpy as np

T = 2048
C = 1024
G = 512          # per-core head-group width (8 heads x 64)
D = 64           # head dim
NH = 8           # heads per core
E = 65           # augmented head width (64 + ones column)
QCH = 512        # query chunk
KBLK = 128       # key block


def _build_nc():
    from collections import deque
    from contextlib import ExitStack

    import concourse.bass as bass
    import concourse.mybir as mybir
    import concourse.tile as tile
    from concourse import bacc

    F32 = mybir.dt.float32
    F32R = mybir.dt.float32r
    BF16 = mybir.dt.bfloat16
    EXP = mybir.ActivationFunctionType.Exp

    nc = bacc.Bacc("TRN2", target_bir_lowering=False)

    # All inputs host-packed so every DMA is one instruction with large
    # per-partition-contiguous runs (big descriptors run at full DMA rate).
    # boot: x columns 0:128 (c-major) ++ all of WV -- the critical startup
    # set in one 10KB/partition-descriptor DMA.
    boot = nc.dram_tensor("boot", [128, 1024 + 8 * G], BF16,
                          kind="ExternalInput")
    # xT: 3 column-waves [0:512],[512:1024],[1024:2048], [128p, 8c, cols]
    # flattened per partition (cols 0:128 duplicated in boot).
    xT = nc.dram_tensor("xT", [128, 8 * T], BF16, kind="ExternalInput")
    wq = nc.dram_tensor("wq", [4, 128, 8 * 128], BF16, kind="ExternalInput")
    wk = nc.dram_tensor("wk", [4, 128, 8 * 128], BF16, kind="ExternalInput")
    wp = nc.dram_tensor("wp", [128, 4 * C], BF16, kind="ExternalInput")
    mask = nc.dram_tensor("mask", [128, 256], BF16, kind="ExternalInput")
    out = nc.dram_tensor("out", [T, C], BF16, kind="ExternalOutput")

    with tile.TileContext(nc) as tc, ExitStack() as ctx:
        persist = ctx.enter_context(tc.tile_pool(name="persist", bufs=1))
        xw = ctx.enter_context(tc.tile_pool(name="xw", bufs=1))
        wsl = ctx.enter_context(tc.tile_pool(name="wsl", bufs=1))
        wqk = ctx.enter_context(tc.tile_pool(name="wqk", bufs=1))
        qtkt = ctx.enter_context(tc.tile_pool(name="qtkt", bufs=2))
        ptp = ctx.enter_context(tc.tile_pool(name="ptp", bufs=6))
        nrm = ctx.enter_context(tc.tile_pool(name="nrm", bufs=2))
        osb = ctx.enter_context(tc.tile_pool(name="osb", bufs=2))
        wpp = ctx.enter_context(tc.tile_pool(name="wpp", bufs=1))
        pss = ctx.enter_context(tc.tile_pool(name="pss", bufs=2, space="PSUM"))
        psy = ctx.enter_context(tc.tile_pool(name="psy", bufs=1, space="PSUM"))
        pfl = ctx.enter_context(tc.tile_pool(name="pfl", bufs=2, space="PSUM"))

        VA = [persist.tile([128, NH * 128], BF16, name=f"va{i}", tag=f"va{i}")
              for i in range(16)]
        YT = [persist.tile([128, T], BF16, name=f"yt{i}", tag=f"yt{i}")
              for i in range(4)]
        MSK = persist.tile([128, 256], BF16, name="msk", tag="msk")

        # Boot DMA (x wave0 + WV fused) first, then xT column-waves.
        BOOT = wsl.tile([128, 1024 + 8 * G], BF16, name="boot", tag="boot")
        nc.sync.dma_start(out=BOOT, in_=boot[:, :])
        WVA = BOOT[:, 1024 : 1024 + 8 * G].rearrange("p (c n) -> p c n", c=8)
        XTA = xw.tile([128, 8, T], BF16, name="xta", tag="xta")
        nc.sync.dma_start(out=XTA[:, :, 0:512], in_=xT[:, 0:4096])
        nc.sync.dma_start(out=XTA[:, :, 512:1024], in_=xT[:, 4096:8192])
        nc.sync.dma_start(out=XTA[:, :, 1024:2048], in_=xT[:, 8192:16384])

        # PE warm-up during the initial DMA wait: ~3us of junk matmuls so
        # the HAM clock gate is at 8/8 when real work arrives.
        junk = persist.tile([128, 512], BF16, name="junk", tag="junk")
        nc.vector.memset(junk, 0.0)
        wps = pfl.tile([128, 512], F32, name="fill", tag="fill")
        for i in range(7):
            nc.tensor.matmul(
                wps, junk[:, 0:128], junk, start=(i == 0), stop=(i == 6)
            )

        # V-augmentation: 64 ones columns per head -> AV matmul emits the
        # softmax denominator replicated over psum rows 64:128 for free.
        for tb in range(16):
            vdst = VA[tb].rearrange("p (h e) -> p h e", e=128)[:, :, 64:128]
            nc.vector.memset(vdst, 1.0)

        # ---------------- phase 0: V ----------------
        for tb in range(16):
            ps = pfl.tile([128, 512], F32, name="fill", tag="fill")
            for c in range(8):
                lhsT = (
                    BOOT[:, c * 128 : (c + 1) * 128]
                    if tb == 0
                    else XTA[:, c, tb * 128 : (tb + 1) * 128]
                )
                nc.tensor.matmul(
                    ps,
                    lhsT,
                    WVA[:, c, :],
                    start=(c == 0),
                    stop=(c == 7),
                )
            vdst = VA[tb].rearrange("p (h e) -> p h e", e=128)[:, :, 0:64]
            nc.vector.tensor_copy(vdst, ps.rearrange("p (h d) -> p h d", d=64))

        nc.sync.dma_start(out=MSK, in_=mask[:, :])

        # Prefetch every pair's Q/K weight slices up-front -- issuing them at
        # pair start leaves the first pumped filler units stalled on the DMA.
        WQK = []
        for hp in range(4):
            tiles = {}
            for mat, dram in (("q", wq), ("k", wk)):
                w = wqk.tile(
                    [128, 8, 128], BF16,
                    name=f"w{mat}{hp}", tag=f"w{mat}{hp}",
                )
                nc.sync.dma_start(out=w, in_=dram[hp, :, :])
                tiles[mat] = w
            WQK.append(tiles)

        WPA = wpp.tile([128, 4, C], BF16, name="wpa", tag="wpa")
        nc.sync.dma_start(out=WPA, in_=wp[:, :])

        # ---------------- QK machinery ----------------
        def make_qk_units(hp):
            wtiles = WQK[hp]
            qt = qtkt.tile([128, T], BF16, name="qtP", tag="qtP")
            kt = qtkt.tile([128, T], BF16, name="ktP", tag="ktP")
            units = []
            for t4 in range(4):
                for mat, dst in (("q", qt), ("k", kt)):
                    st = {}
                    def unit_a(mat=mat, t4=t4, st=st):
                        st["ps"] = pfl.tile(
                            [128, 512], F32, name="fill", tag="fill"
                        )
                        for c in range(4):
                            nc.tensor.matmul(
                                st["ps"],
                                wtiles[mat][:, c, :],
                                XTA[:, c, t4 * 512 : (t4 + 1) * 512],
                                start=(c == 0),
                                stop=False,
                            )
                    def unit_b(mat=mat, dst=dst, t4=t4, st=st):
                        for c in range(4, 8):
                            nc.tensor.matmul(
                                st["ps"],
                                wtiles[mat][:, c, :],
                                XTA[:, c, t4 * 512 : (t4 + 1) * 512],
                                start=False,
                                stop=(c == 7),
                            )
                        nc.vector.tensor_copy(
                            dst[:, t4 * 512 : (t4 + 1) * 512], st["ps"]
                        )
                    units.append(unit_a)
                    units.append(unit_b)
            return qt, kt, units

        # ---------- proj units (tail / fillers for pair 3) ----------
        def proj_units(tb):
            ot = {}
            def unit_ch(ch):
                def unit():
                    if ch == 0:
                        ot["t"] = osb.tile([128, C], BF16, name="ot", tag="ot")
                    ps = pfl.tile([128, 512], F32, name="fill", tag="fill")
                    for cb in range(4):
                        nc.tensor.matmul(
                            ps,
                            YT[cb][:, tb * 128 : (tb + 1) * 128],
                            WPA[:, cb, ch * 512 : (ch + 1) * 512],
                            start=(cb == 0),
                            stop=(cb == 3),
                        )
                    nc.vector.tensor_copy(
                        ot["t"][:, ch * 512 : (ch + 1) * 512], ps
                    )
                    nc.sync.dma_start(
                        out=out[
                            tb * 128 : (tb + 1) * 128,
                            ch * 512 : (ch + 1) * 512,
                        ],
                        in_=ot["t"][:, ch * 512 : (ch + 1) * 512],
                    )
                return unit
            return [unit_ch(0), unit_ch(1)]

        def tail_units(qc):
            units = []
            for tb in range(qc * 4, qc * 4 + 4):
                units.extend(proj_units(tb))
            return units

        # ---------------- attention ----------------
        fill_q = deque()

        def pump(n):
            for _ in range(min(n, len(fill_q))):
                fill_q.popleft()()

        def attention(hp, qt, kt, qc):
            q0 = qc * QCH
            nkb = (qc + 1) * 4
            hA, hB = 2 * hp, 2 * hp + 1
            ytA = psy.tile([128, QCH], F32, name="ytA", tag="ytA")
            ytB = psy.tile([128, QCH], F32, name="ytB", tag="ytB")

            def emit_av_h(sub, kb, pt, off, w):
                yt = ytA if sub == 0 else ytB
                h = hA if sub == 0 else hB
                nc.tensor.matmul(
                    yt[:, off : off + w],
                    VA[kb][:, h * 128 : h * 128 + 128],
                    pt[:, sub * 512 + off : sub * 512 + off + w],
                    start=(kb == 0),
                    stop=(kb == nkb - 1),
                )

            def emit_av(kb, pt, off, w):
                emit_av_h(0, kb, pt, off, w)
                emit_av_h(1, kb, pt, off, w)

            def emit_s(kb):
                j = kb - qc * 4
                off = j * 128 if j >= 1 else 0
                ksl = slice(kb * KBLK, (kb + 1) * KBLK)
                sAB = pss.tile([128, 1024], F32, name="sAB", tag="sAB")
                nc.tensor.matmul(
                    sAB[:, off : 512],
                    kt[0:64, ksl],
                    qt[0:64, q0 + off : q0 + QCH],
                    start=True,
                    stop=True,
                    tile_position=(0, 0),
                )
                nc.tensor.matmul(
                    sAB[:, 512 + off : 1024],
                    kt[64:128, ksl],
                    qt[64:128, q0 + off : q0 + QCH],
                    start=True,
                    stop=True,
                    tile_position=(64, 0),
                )
                return kb, sAB, off

            def emit_exp(kb, sAB, off):
                j = kb - qc * 4
                pt = ptp.tile([128, 1024], BF16, name="pt", tag="pt")
                if j >= 1:
                    nc.scalar.activation(
                        pt[:, off:512], sAB[:, off:512], EXP, scale=0.125
                    )
                    nc.scalar.activation(
                        pt[:, 512 + off : 1024],
                        sAB[:, 512 + off : 1024],
                        EXP,
                        scale=0.125,
                    )
                else:
                    nc.scalar.activation(pt, sAB, EXP, scale=0.125)
                if j >= 0:
                    pv = pt.rearrange("p (s q) -> p s q", s=2)[
                        :, :, off : off + 128
                    ]
                    nc.vector.tensor_mul(
                        pv, pv, MSK.rearrange("p (s q) -> p s q", s=2)
                    )
                return (kb, pt, off, 512 - off)

            pend = deque()
            for kbp in range(0, nkb, 2):
                s0 = emit_s(kbp)
                s1 = emit_s(kbp + 1)
                e0 = emit_exp(*s0)
                e1 = emit_exp(*s1)
                pump(2)
                if len(pend) == 4:
                    emit_av(*pend.popleft())
                    emit_av(*pend.popleft())
                pend.append(e0)
                pend.append(e1)
            # Drain per head: all A-AVs, then A's norm (DVE) overlapping
            # B's AV matmuls on PE.
            plist = list(pend)
            pend.clear()

            def norm_sub(sub, yt):
                lsb = nrm.tile([64, 512], F32, name="lsb", tag="lsb")
                nc.vector.tensor_copy(lsb, yt[64:128, :])
                lrec = nrm.tile([64, 512], F32, name="lrec", tag="lrec")
                nc.vector.reciprocal_approx_fast(lrec, lsb)
                nblk = 4 if (hp == 3 and qc == 3) else 1
                bw = QCH // nblk
                for blk in range(nblk):
                    sl = slice(blk * bw, (blk + 1) * bw)
                    nc.vector.tensor_mul(
                        YT[hp][
                            sub * 64 : (sub + 1) * 64,
                            q0 + blk * bw : q0 + (blk + 1) * bw,
                        ],
                        yt[0:64, sl],
                        lrec[:, sl],
                    )

            for ent in plist:
                emit_av_h(0, *ent)
            norm_sub(0, ytA)
            for ent in plist:
                emit_av_h(1, *ent)
            norm_sub(1, ytB)

        # ---------------- main schedule ----------------
        qt, kt, units = make_qk_units(0)
        for u in units:
            u()
        for hp in range(4):
            nqt = nkt = None
            if hp < 3:
                nqt, nkt, nunits = make_qk_units(hp + 1)
                fill_q.extend(nunits)
            for qc in range(4):
                if hp == 3 and qc >= 1:
                    fill_q.extend(tail_units(qc - 1))
                attention(hp, qt, kt, qc)
                pump(2)
            # keep a few units in the queue so the next pair's early
            # ACT-paced iterations still have PE filler
            pump(len(fill_q) - 4 if hp < 3 else len(fill_q))
            if hp < 3:
                qt, kt = nqt, nkt
        for u in tail_units(3):
            u()

    nc.compile()
    return nc


_NC_CACHE = None


def kernel(x0, w_attn, w_proj, _trace=False, _tmpdir=None):
    global _NC_CACHE
    import ml_dtypes

    from concourse.bass_utils import run_bass_kernel_spmd

    BF = ml_dtypes.bfloat16
    x0 = np.asarray(x0, dtype=np.float32)
    w_attn = np.asarray(w_attn, dtype=np.float32)
    w_proj = np.asarray(w_proj, dtype=np.float32)
    B = x0.shape[0]

    if _NC_CACHE is None:
        _NC_CACHE = _build_nc()
    nc = _NC_CACHE

    tri = np.triu(np.ones((128, 128), dtype=np.float32))
    msk = np.concatenate([tri, tri], axis=1).astype(BF)

    def pack_x(xb):
        # [C, T] -> [128p, 8c, T] -> 3 column-waves flattened per partition
        xw = xb.T.reshape(8, 128, T).transpose(1, 0, 2)
        waves = [xw[:, :, 0:512], xw[:, :, 512:1024], xw[:, :, 1024:2048]]
        return np.ascontiguousarray(
            np.concatenate([w.reshape(128, -1) for w in waves], axis=1)
        ).astype(BF)

    def pack_boot(xb, wvg):
        # x cols 0:128 (c-major per partition) ++ WV (c-major per partition)
        xw = xb.T.reshape(8, 128, T).transpose(1, 0, 2)[:, :, 0:128]
        wvw = wvg.reshape(8, 128, G).transpose(1, 0, 2)
        return np.ascontiguousarray(
            np.concatenate(
                [xw.reshape(128, -1), wvw.reshape(128, -1)], axis=1
            )
        ).astype(BF)

    def pack_qk(wm):
        # [C, G] -> [4hp, 128p, 8c*128] per-pair contiguous
        return np.ascontiguousarray(
            wm.reshape(8, 128, 4, 128).transpose(2, 1, 0, 3).reshape(
                4, 128, 1024
            )
        ).astype(BF)

    in_maps = []
    for core in range(8):
        b, g = divmod(core, 2)
        wvg = w_attn[:, 2 * C + g * G : 2 * C + (g + 1) * G]
        wpg = w_proj[g * G : (g + 1) * G, :]
        in_maps.append(
            {
                "boot": pack_boot(x0[b], wvg),
                "xT": pack_x(x0[b]),
                "wq": pack_qk(w_attn[:, g * G : (g + 1) * G]),
                "wk": pack_qk(w_attn[:, C + g * G : C + (g + 1) * G]),
                "wp": np.ascontiguousarray(
                    wpg.reshape(4, 128, C).transpose(1, 0, 2).reshape(
                        128, 4 * C
                    )
                ).astype(BF),
                "mask": msk,
            }
        )

    res = run_bass_kernel_spmd(
        nc, in_maps, list(range(8)), trace=_trace, tmpdir=_tmpdir
    )
    outp = np.empty((B, T, C), dtype=np.float32)
    for b in range(B):
        outp[b] = res.results[2 * b]["out"].astype(np.float32) + res.results[
            2 * b + 1
        ]["out"].astype(np.float32)
    if _trace:
        kernel.last_exec_time_ns = res.exec_time_ns
    return outp

